# revision 18
# baseline (speedup 1.0000x reference)
"""GRU actor (B=1024, T=512, D=64, H=256) on 8 TRN2 NeuronCores.

Pure data parallel: each core owns 128 batch rows, weights replicated.
Everything on-chip is kept in *transposed* layout ([feature, batch]) so the
sequential GRU recurrence needs no transposes:
  gates[3H, B] = W_hh @ h[H, B]  +  W_ih_aug @ x_aug[D+1, B]
with the x-side biases folded into an augmented ones-row of x.

The per-core batch of 128 is split into two independent 64-column chains so
the per-step serial dependency (PE -> ACT -> DVE -> ACT -> DVE -> PE) of one
chain overlaps the other chain's work on different engines. GPSIMD (Pool)
takes part of the elementwise combine to unload VectorE.
bf16 matmuls (f32 PSUM accumulate), ScalarE sigmoid/tanh.

The tensor engine's DVFS runs matmuls at 1.2GHz unless it sees ~3us of
continuous activity, and the fast (~2.4GHz) state decays after ~100us of
bursty use. A one-time warmup burst plus 12 dependency-free dummy matmuls
per timestep (into a dedicated junk PSUM bank, draining in the elementwise
window) hold the fast state for the whole kernel: 53ns -> 29ns per 64-col
matmul on the critical rz-matmul block (~-200ns/step).
"""

import numpy as np
import ml_dtypes

LAST_RESULTS = None

import concourse.mybir as mybir
from concourse import bass, bacc
from concourse.tile import TileContext
from concourse.bass_utils import run_bass_kernel_spmd

BF = mybir.dt.bfloat16
F32 = mybir.dt.float32
AF = mybir.ActivationFunctionType

B, T, D, H = 1024, 512, 64, 256
NCORES = 8
BC = B // NCORES  # 128 batch rows per core
G = 6  # 3H/128 gate tiles: 0,1=r  2,3=z  4,5=n
XBLK = 8  # timesteps per x DMA block
NCH = 2  # independent batch chains per core
CW = BC // NCH  # chain width (64)


def build_nc():
    nc = bacc.Bacc()

    xt = nc.declare_dram_parameter("xt", [D + 1, T, BC], BF, isOutput=False)
    wih = nc.declare_dram_parameter("wih", [D + 1, G, 128], BF, isOutput=False)
    whh = nc.declare_dram_parameter("whh", [128, 2, G, 128], BF, isOutput=False)
    bhn = nc.declare_dram_parameter("bhn", [1, 2, 128], BF, isOutput=False)
    wbase = nc.declare_dram_parameter("wbase", [128, 2, 2, 128], BF, isOutput=False)
    bbase = nc.declare_dram_parameter("bbase", [1, 2, 128], BF, isOutput=False)
    wdir = nc.declare_dram_parameter("wdir", [128, 2, 8], BF, isOutput=False)
    wmag = nc.declare_dram_parameter("wmag", [128, 2, 8], BF, isOutput=False)
    bdm = nc.declare_dram_parameter("bdm", [1, 2, 8], BF, isOutput=False)
    inx = nc.declare_dram_parameter("inx", [128, T, 2, BC], BF, isOutput=False)
    out = nc.declare_dram_parameter("out", [8, BC], F32, isOutput=True)

    with TileContext(nc) as tc:
        with (
            tc.tile_pool(name="const", bufs=1) as cpool,
            tc.tile_pool(name="xpool", bufs=3) as xpool,
            tc.tile_pool(name="state", bufs=1) as spool,
            tc.tile_pool(name="work", bufs=4) as wpool,
            tc.tile_pool(name="psum", bufs=2, space="PSUM") as ppool,
            tc.tile_pool(name="nipsum", bufs=1, space="PSUM") as nipool,
            tc.tile_pool(name="jpsum", bufs=1, space="PSUM") as jpool,
        ):
            wih_sb = cpool.tile([D + 1, G, 128], BF)
            nc.sync.dma_start(out=wih_sb[:], in_=wih[:])
            whh_sb = cpool.tile([128, 2, G, 128], BF)
            nc.sync.dma_start(out=whh_sb[:], in_=whh[:])
            bhn_sb = cpool.tile([1, 2, 128], BF)
            nc.sync.dma_start(out=bhn_sb[:], in_=bhn[:])
            wbase_sb = cpool.tile([128, 2, 2, 128], BF)
            nc.sync.dma_start(out=wbase_sb[:], in_=wbase[:])
            bbase_sb = cpool.tile([1, 2, 128], BF)
            nc.sync.dma_start(out=bbase_sb[:], in_=bbase[:])
            wdir_sb = cpool.tile([128, 2, 8], BF)
            nc.sync.dma_start(out=wdir_sb[:], in_=wdir[:])
            wmag_sb = cpool.tile([128, 2, 8], BF)
            nc.sync.dma_start(out=wmag_sb[:], in_=wmag[:])
            bdm_sb = cpool.tile([1, 2, 8], BF)
            nc.sync.dma_start(out=bdm_sb[:], in_=bdm[:])

            ones_sb = cpool.tile([1, BC], BF)
            nc.vector.memset(ones_sb[:], 1.0)

            # PE p-state warmup: DVFS steps the tensor engine from 1.2GHz to
            # ~2.4GHz only after ~3us of CONTINUOUS matmul activity, and the
            # fast state then persists across the per-step idle gaps. Without
            # this burst every 64-col matmul costs 53ns instead of ~34ns.
            junk_ps = jpool.tile([128, 256], F32, tag="junk", name="junk")
            for i in range(128):
                nc.tensor.matmul(
                    junk_ps[:, :64], whh_sb[:, 0, i % G], whh_sb[:, 1, i % G, :64],
                    start=True, stop=True, skip_group_check=True,
                )

            # per-chain recurrent state, [H, chain-batch] transposed
            h_ch = []
            for c in range(NCH):
                h_c = spool.tile([128, 2, CW], BF, tag=f"h{c}")
                nc.vector.memset(h_c[:], 0.0)
                h_ch.append(h_c)

            for blk in range(T // XBLK):
                xt_sb = xpool.tile([D + 1, XBLK, BC], BF)
                nc.sync.dma_start(
                    out=xt_sb[:], in_=xt[:, blk * XBLK : (blk + 1) * XBLK, :]
                )
                inx_sb = xpool.tile([128, XBLK, 2, BC], BF)
                nc.sync.dma_start(
                    out=inx_sb[:], in_=inx[:, blk * XBLK : (blk + 1) * XBLK]
                )
                for j in range(XBLK):
                    # Separate PSUM tiles (= separate banks) for rz and ni so
                    # the r/z sigmoid's dependency set is ONLY the rz matmuls
                    # (tile-granular dep tracking: merging rz+ni in one bank
                    # makes the sigmoid wait on the ni matmuls too, +170ns).
                    # ps_ni from a bufs=1 pool: its first writer each step
                    # (bias matmul, start=True) WAR-waits on the previous
                    # step's mul — off the critical path.
                    ps_rz_c, ps_ni_c = [], []
                    for c in range(NCH):
                        ps_rz_c.append(ppool.tile([128, 4, CW], F32, tag=f"ps_rz{c}", name=f"ps_rz{c}"))
                        ps_ni_c.append(nipool.tile([128, 2, CW], F32, tag=f"ps_ni{c}", name=f"ps_ni{c}"))

                    # ---- PE: gate matmuls, chain-interleaved ----
                    for c in range(NCH):
                        lo, hi = c * CW, (c + 1) * CW
                        h_c = h_ch[c]
                        xcol = xt_sb[:, j, lo:hi]
                        onesc = ones_sb[:, lo:hi]
                        ps_rz = ps_rz_c[c]
                        ps_hn = ps_ni_c[c]
                        # x-only matmuls first: PE (in-order) hoists these
                        # into the previous step's elementwise window.
                        for g in range(4):
                            nc.tensor.matmul(
                                ps_rz[:, g], wih_sb[:, g], xcol,
                                start=(g == 0), stop=False, skip_group_check=True,
                            )
                        for g2 in range(2):
                            nc.tensor.matmul(
                                ps_hn[:, g2], bhn_sb[:, g2], onesc,
                                start=(g2 == 0), stop=False, skip_group_check=True,
                            )
                        # h-dependent matmuls: r gates (g0,g1) fully first so
                        # the r-sigmoid fires after only 4 MMs
                        for g in range(4):
                            nc.tensor.matmul(
                                ps_rz[:, g], whh_sb[:, 0, g], h_c[:, 0],
                                start=False, stop=False, skip_group_check=True,
                            )
                            nc.tensor.matmul(
                                ps_rz[:, g], whh_sb[:, 1, g], h_c[:, 1],
                                start=False, stop=False, skip_group_check=True,
                            )
                        for g2 in range(2):
                            nc.tensor.matmul(
                                ps_hn[:, g2], whh_sb[:, 0, 4 + g2], h_c[:, 0],
                                start=False, stop=False, skip_group_check=True,
                            )
                        for g2 in range(2):
                            nc.tensor.matmul(
                                ps_hn[:, g2], whh_sb[:, 1, 4 + g2], h_c[:, 1],
                                start=False, stop=(g2 == 1), skip_group_check=True,
                            )

                    # ---- elementwise, phase-interleaved across chains so no
                    # engine head-of-line-blocks the other chain's ready op ----
                    rz_sb_c = [wpool.tile([128, 4, CW], BF, tag=f"rz{c}", name=f"rz{c}") for c in range(NCH)]
                    for c in range(NCH):
                        nc.scalar.activation(rz_sb_c[c][:], ps_rz_c[c][:], AF.Sigmoid)
                    rhn_c = [wpool.tile([128, 2, CW], BF, tag=f"rhn{c}", name=f"rhn{c}") for c in range(NCH)]
                    for c in range(NCH):
                        nc.vector.tensor_mul(rhn_c[c][:], rz_sb_c[c][:, 0:2], ps_ni_c[c][:])
                    npre_c = [wpool.tile([128, 2, CW], BF, tag=f"npre{c}", name=f"npre{c}") for c in range(NCH)]
                    for c in range(NCH):
                        nc.vector.tensor_add(npre_c[c][:], rhn_c[c][:], inx_sb[:, j, :, c * CW : (c + 1) * CW])
                    # zh = z*h on Pool, off the critical path (needs only sigmoid+old h)
                    zh_c = [wpool.tile([128, 2, CW], BF, tag=f"zh{c}", name=f"zh{c}") for c in range(NCH)]
                    for c in range(NCH):
                        nc.gpsimd.tensor_mul(zh_c[c][:], rz_sb_c[c][:, 2:4], h_ch[c][:])
                    n_sb_c = [wpool.tile([128, 2, CW], BF, tag=f"n{c}", name=f"n{c}") for c in range(NCH)]
                    for c in range(NCH):
                        nc.scalar.activation(n_sb_c[c][:], npre_c[c][:], AF.Tanh)
                    # zc = 1 - z off the critical path (4x imm tensor_scalar)
                    zc_c = [wpool.tile([128, 2, CW], BF, tag=f"zc{c}", name=f"zc{c}") for c in range(NCH)]
                    for c in range(NCH):
                        nc.gpsimd.tensor_scalar(
                            zc_c[c][:], rz_sb_c[c][:, 2:4], -1.0, 1.0,
                            op0=mybir.AluOpType.mult, op1=mybir.AluOpType.add,
                        )
                    # h_new = zc*n + zh ; two 2x DVE ops post-tanh
                    t1_c = [wpool.tile([128, 2, CW], BF, tag=f"t1{c}", name=f"t1{c}") for c in range(NCH)]
                    for c in range(NCH):
                        nc.vector.tensor_mul(t1_c[c][:], zc_c[c][:], n_sb_c[c][:])
                    for c in range(NCH):
                        nc.vector.tensor_add(h_ch[c][:], zh_c[c][:], t1_c[c][:])

                    # PE DVFS gap filler: dep-free 256-col dummy matmuls after
                    # this step's real matmuls keep the tensor engine near-
                    # continuously busy so it holds the fast p-state (53ns ->
                    # 34ns per 64-col matmul). They drain during the element-
                    # wise window, before the next step's h-matmuls are ready.
                    for i in range(12):
                        nc.tensor.matmul(
                            junk_ps[:], whh_sb[:, 0, i % G], whh_sb[:, 1, 0:2],
                            start=True, stop=True, skip_group_check=True,
                        )

            # ---- head MLP on h_T (full width, once) ----
            ones = ones_sb[:]
            ps_base = ppool.tile([128, 2, BC], F32, tag="ps_rz0")
            for c in range(NCH):
                lo, hi = c * CW, (c + 1) * CW
                for mm in range(2):
                    nc.tensor.matmul(
                        ps_base[:, mm, lo:hi], wbase_sb[:, 0, mm], h_ch[c][:, 0],
                        start=True, stop=False,
                    )
                    nc.tensor.matmul(
                        ps_base[:, mm, lo:hi], wbase_sb[:, 1, mm], h_ch[c][:, 1],
                        start=False, stop=False,
                    )
                    nc.tensor.matmul(
                        ps_base[:, mm, lo:hi], bbase_sb[:, mm], ones[:, lo:hi],
                        start=False, stop=True,
                    )
            base_sb = wpool.tile([128, 2, BC], BF, tag="base")
            nc.scalar.activation(base_sb[:], ps_base[:], AF.Relu)

            ps_dm = ppool.tile([8, 2, BC], F32, tag="ps_rz1")
            for which, w_sb in ((0, wdir_sb), (1, wmag_sb)):
                nc.tensor.matmul(
                    ps_dm[:, which], w_sb[:, 0], base_sb[:, 0],
                    start=True, stop=False,
                )
                nc.tensor.matmul(
                    ps_dm[:, which], w_sb[:, 1], base_sb[:, 1],
                    start=False, stop=False,
                )
                nc.tensor.matmul(
                    ps_dm[:, which], bdm_sb[:, which], ones, start=False, stop=True
                )
            dir_sb = wpool.tile([8, BC], BF, tag="dir")
            nc.scalar.activation(dir_sb[:], ps_dm[:, 0], AF.Tanh)
            mag_sb = wpool.tile([8, BC], BF, tag="mag")
            nc.scalar.activation(mag_sb[:], ps_dm[:, 1], AF.Sigmoid)
            outf = wpool.tile([8, BC], F32, tag="outf")
            nc.vector.tensor_mul(outf[:], dir_sb[:], mag_sb[:])
            nc.sync.dma_start(out=out[:], in_=outf[:])

    nc.compile()
    return nc


def _prep_shared(w_ih, w_hh, b_ih, b_hh, w_base, b_base, w_dir, b_dir, w_mag, b_mag):
    bf = ml_dtypes.bfloat16
    # augmented [D+1, 3H] input weights: last row carries the x-side biases
    wih_aug = np.zeros((D + 1, 3 * H), np.float32)
    wih_aug[:D] = w_ih.T
    brow = b_ih.astype(np.float32).copy()
    brow[: 2 * H] += b_hh[: 2 * H]  # r,z biases combine; n keeps b_hh separate
    wih_aug[D] = brow
    wih_p = wih_aug.reshape(D + 1, G, 128).astype(bf)

    whh_p = (
        w_hh.reshape(G, 128, 2, 128).transpose(3, 2, 0, 1).astype(bf)
    )  # [p, kk, g, m] = w_hh[g*128+m, kk*128+p]
    bhn_p = b_hh[2 * H :].reshape(1, 2, 128).astype(bf)
    wbase_p = w_base.reshape(2, 128, 2, 128).transpose(3, 2, 0, 1).astype(bf)
    bbase_p = b_base.reshape(1, 2, 128).astype(bf)
    wdir_p = w_dir.T.reshape(2, 128, 8).transpose(1, 0, 2).astype(bf)
    wmag_p = w_mag.T.reshape(2, 128, 8).transpose(1, 0, 2).astype(bf)
    bdm_p = np.stack([b_dir, b_mag]).reshape(1, 2, 8).astype(bf)
    return dict(
        wih=wih_p, whh=whh_p, bhn=bhn_p, wbase=wbase_p, bbase=bbase_p,
        wdir=wdir_p, wmag=wmag_p, bdm=bdm_p,
    )


def _prep_inx(x_shard, w_ih, b_ih):
    # host-precomputed n-gate x-projection: i_n = x @ w_ih_n.T + b_ih_n
    gi_n = x_shard.reshape(-1, D).astype(np.float32) @ w_ih[2 * H :].T.astype(np.float32)
    gi_n += b_ih[2 * H :]
    # [Bc, T, 256] -> [128(p), T, 2(kk), Bc]
    return (
        gi_n.reshape(BC, T, 2, 128).transpose(3, 1, 2, 0).astype(ml_dtypes.bfloat16)
    )


def kernel(x_seq, w_ih, w_hh, b_ih, b_hh, w_base, b_base, w_dir, b_dir,
           w_mag, b_mag, _trace=False, _tmpdir=None):
    bf = ml_dtypes.bfloat16
    shared = _prep_shared(
        w_ih, w_hh, b_ih, b_hh, w_base, b_base, w_dir, b_dir, w_mag, b_mag
    )
    ones_row = np.ones((1, T, BC), np.float32)
    in_maps = []
    for i in range(NCORES):
        shard = x_seq[i * BC : (i + 1) * BC]  # [128, 512, 64]
        xt_i = np.concatenate(
            [shard.transpose(2, 1, 0), ones_row], axis=0
        ).astype(bf)  # [65, 512, 128]
        m = dict(shared)
        m["xt"] = xt_i
        m["inx"] = _prep_inx(shard, w_ih, b_ih)
        in_maps.append(m)

    nc = build_nc()
    res = run_bass_kernel_spmd(
        nc, in_maps, core_ids=list(range(NCORES)),
        trace=_trace, tmpdir=_tmpdir,
    )
    global LAST_RESULTS
    LAST_RESULTS = res
    out_full = np.empty((B, 8), np.float32)
    for i in range(NCORES):
        out_full[i * BC : (i + 1) * BC] = res.results[i]["out"].T
    return out_full

